# revision 6
# baseline (speedup 1.0000x reference)
"""MoE dispatch/combine kernel for Trainium2 (8 NeuronCores, token-parallel).

Computes, for hidden_states [B=4, S=4096, H=2048], router_weight [E=64, H],
router_bias [E], expert_bias [E, H], TOP_K=8:

    logits = x @ rw.T + rb ; scores = softmax(logits) ; top8
    out = x * (sum top8 scores) + (top8-masked scores) @ expert_bias

Strategy (per core, 2048 tokens):
  - x tiles [128, 2048] streamed in as float32r.
  - PE transposes x chunks -> xT; router matmul (fp32r, rwT stationary)
    accumulates logitsT [64, 512] per 512-token block; +bias on PSUM->SBUF copy.
  - PE transposes logits back to [128 tok, 64 exp]; DVE Max8 gives the top-8
    values directly; ACT exp with per-token bias and accumulated row-sums
    gives softmax pieces; one chained tensor_scalar builds the top8 mask.
  - Combine = C @ eb and x*a = diag(a) @ x, both fp32r matmuls accumulating
    into the same PSUM bank; ACT copies out; DMA back.
No collectives needed (pure data parallel over tokens).
"""
import os
import sys

for _p in ("/opt/trn_rl_repo", "/opt/pypackages"):
    if _p not in sys.path:
        sys.path.append(_p)

# Never let an env-set BASS_TRACE crash the run on the missing axon hook.
os.environ.setdefault("BASS_NEVER_TRACE", "1")

import numpy as np
from contextlib import ExitStack

import concourse.bass as bass
import concourse.tile as tile
from concourse import bacc, mybir
from concourse.bass_utils import run_bass_kernel_spmd

F32 = mybir.dt.float32
F32R = mybir.dt.float32r
AF = mybir.ActivationFunctionType
AL = mybir.AluOpType

B, S, H, E, TOPK = 4, 4096, 2048, 64, 8
T = B * S                      # 16384 tokens
N_CORES = 8
T_PC = T // N_CORES            # 2048 tokens per core
BLK = 256                      # tokens per block
N_BLK = T_PC // BLK            # 8
TPB = BLK // 128               # 2 tiles of 128 tokens per block
HCH = H // 128                 # 16 h-chunks


def _build():
    nc = bacc.Bacc("TRN2", target_bir_lowering=False, debug=False,
                   num_devices=N_CORES)

    x_d = nc.dram_tensor("x", [T_PC, H], F32R, kind="ExternalInput").ap()
    rwt_d = nc.dram_tensor("rwt", [H, E], F32R, kind="ExternalInput").ap()
    eb_d = nc.dram_tensor("eb", [E, H], F32R, kind="ExternalInput").ap()
    rb_d = nc.dram_tensor("rb", [E, 1], F32, kind="ExternalInput").ap()
    idr_d = nc.dram_tensor("idr", [128, 128], F32R, kind="ExternalInput").ap()
    idf_d = nc.dram_tensor("idf", [128, 128], F32, kind="ExternalInput").ap()
    out_d = nc.dram_tensor("out", [T_PC, H], F32, kind="ExternalOutput").ap()

    with tile.TileContext(nc) as tc:
        with ExitStack() as ctx:
            consts = ctx.enter_context(tc.tile_pool(name="consts", bufs=1))
            xp = ctx.enter_context(tc.tile_pool(name="xp", bufs=4))
            xtp = ctx.enter_context(tc.tile_pool(name="xtp", bufs=2))
            lgp = ctx.enter_context(tc.tile_pool(name="lgp", bufs=2))
            wp = ctx.enter_context(tc.tile_pool(name="wp", bufs=2))
            yp = ctx.enter_context(tc.tile_pool(name="yp", bufs=2))
            stp = ctx.enter_context(tc.tile_pool(name="stp", bufs=2))
            cp = ctx.enter_context(tc.tile_pool(name="cp", bufs=3))
            op = ctx.enter_context(tc.tile_pool(name="op", bufs=2))

            tp_ps = ctx.enter_context(
                tc.tile_pool(name="tp_ps", bufs=2, space="PSUM"))
            lg_ps = ctx.enter_context(
                tc.tile_pool(name="lg_ps", bufs=2, space="PSUM"))
            w_ps = ctx.enter_context(
                tc.tile_pool(name="w_ps", bufs=1, space="PSUM"))
            ct_ps = ctx.enter_context(
                tc.tile_pool(name="ct_ps", bufs=1, space="PSUM"))
            out_ps = ctx.enter_context(
                tc.tile_pool(name="out_ps", bufs=2, space="PSUM"))

            # ---- constants ----
            rwt = consts.tile([128, HCH, E], F32R)
            nc.sync.dma_start(rwt[:], rwt_d.rearrange("(c p) e -> p c e", p=128))
            eb = consts.tile([E, H], F32R)
            nc.sync.dma_start(eb[:], eb_d)
            rb = consts.tile([E, 1], F32)
            nc.sync.dma_start(rb[:], rb_d)
            idr = consts.tile([128, 128], F32R)
            nc.sync.dma_start(idr[:], idr_d)
            idf = consts.tile([128, 128], F32)
            nc.sync.dma_start(idf[:], idf_d)

            for b in range(N_BLK):
                t0 = b * BLK
                # ---- load x tiles; transpose each as soon as it lands ----
                xs = []
                xT = xtp.tile([128, HCH, BLK], F32R)
                for j in range(TPB):
                    xt = xp.tile([128, H], F32R, tag=f"x{j}")
                    nc.sync.dma_start(xt[:], x_d[t0 + 128 * j:t0 + 128 * (j + 1), :])
                    xs.append(xt)
                    # 16 chunk transposes of tile j, staged 4 per PSUM bank
                    for c0 in range(0, HCH, 4):
                        tp = tp_ps.tile([128, 512], F32R, tag="tp")
                        for ci in range(4):
                            c = c0 + ci
                            nc.tensor.matmul(
                                tp[:, 128 * ci:128 * (ci + 1)],
                                xt[:, 128 * c:128 * (c + 1)],
                                idr[:], is_transpose=True,
                                start=(ci == 0), stop=(ci == 3))
                        nc.any.tensor_copy(
                            xT[:, c0:c0 + 4, 128 * j:128 * (j + 1)], tp[:])

                # ---- router matmul: logitsT [E, BLK] ----
                lg = lg_ps.tile([E, BLK], F32, tag="lg")
                for c in range(HCH):
                    nc.tensor.matmul(lg[:], rwt[:, c, :], xT[:, c, :],
                                     start=(c == 0), stop=(c == HCH - 1))
                # +router bias while copying PSUM -> SBUF
                lgs = lgp.tile([E, BLK], F32)
                nc.scalar.activation(lgs[:], lg[:], AF.Identity,
                                     bias=rb[:], scale=1.0)

                # ---- transpose logits to [128 tok, E] per tile ----
                wps = w_ps.tile([128, TPB * E], F32, tag="wps")
                for j in range(TPB):
                    nc.tensor.matmul(
                        wps[:, E * j:E * (j + 1)],
                        lgs[:, 128 * j:128 * (j + 1)],
                        idf[0:E, 0:E], is_transpose=True,
                        start=(j == 0), stop=(j == TPB - 1))
                w = wp.tile([128, TPB, E], F32)
                nc.any.tensor_copy(w[:], wps[:])

                # ---- softmax + top8 stats ----
                top8 = stp.tile([128, TPB, TOPK], F32, tag="top8")
                for j in range(TPB):
                    nc.vector.max(top8[:, j, :], w[:, j, :])
                negm = stp.tile([128, TPB], F32, tag="negm")
                nc.vector.tensor_scalar(negm[:], top8[:, :, 0], -1.0, None, AL.mult)
                y = yp.tile([128, TPB, E], F32)
                z = stp.tile([128, TPB], F32, tag="z")
                e8 = stp.tile([128, TPB, TOPK], F32, tag="e8")
                s8 = stp.tile([128, TPB], F32, tag="s8")
                for j in range(TPB):
                    nc.scalar.activation(y[:, j, :], w[:, j, :], AF.Exp,
                                         bias=negm[:, j:j + 1], scale=1.0,
                                         accum_out=z[:, j:j + 1])
                    nc.scalar.activation(e8[:, j, :], top8[:, j, :], AF.Exp,
                                         bias=negm[:, j:j + 1], scale=1.0,
                                         accum_out=s8[:, j:j + 1])
                iz = stp.tile([128, TPB], F32, tag="iz")
                nc.vector.reciprocal(iz[:], z[:])
                a = stp.tile([128, TPB], F32, tag="a")
                nc.vector.tensor_tensor(a[:], s8[:], iz[:], op=AL.mult)

                # ---- per tile: mask, combine + scale matmuls, store ----
                for j in range(TPB):
                    g = cp.tile([128, E], F32, tag="g")
                    nc.vector.tensor_scalar(g[:], w[:, j, :],
                                            top8[:, j, TOPK - 1:TOPK],
                                            iz[:, j:j + 1], AL.is_ge, AL.mult)
                    c_t = cp.tile([128, E], F32R, tag="c")
                    nc.vector.tensor_tensor(c_t[:], y[:, j, :], g[:], op=AL.mult)
                    ct = ct_ps.tile([E, 128], F32R, tag="ct")
                    nc.tensor.matmul(ct[:], c_t[:], idr[:], is_transpose=True,
                                     start=True, stop=True)
                    cts = cp.tile([E, 128], F32R, tag="cts")
                    nc.any.tensor_copy(cts[:], ct[:])
                    diag = cp.tile([128, 128], F32R, tag="diag")
                    nc.vector.tensor_scalar(diag[:], idr[:], a[:, j:j + 1],
                                            None, AL.mult)

                    o = op.tile([128, H], F32, tag="o")
                    OW = 512
                    for k in range(H // OW):
                        ops_ = out_ps.tile([128, OW], F32, tag="ops")
                        nc.tensor.matmul(ops_[:], cts[:],
                                         eb[:, OW * k:OW * (k + 1)],
                                         start=True, stop=False)
                        nc.tensor.matmul(ops_[:], diag[:],
                                         xs[j][:, OW * k:OW * (k + 1)],
                                         start=False, stop=True)
                        nc.any.tensor_copy(o[:, OW * k:OW * (k + 1)], ops_[:])
                    nc.sync.dma_start(
                        out_d[t0 + 128 * j:t0 + 128 * (j + 1), :], o[:])

    nc.compile()
    return nc


_NC_CACHE = None


def _get_nc():
    global _NC_CACHE
    if _NC_CACHE is None:
        _NC_CACHE = _build()
    return _NC_CACHE


def kernel(hidden_states, router_weight, router_bias, expert_bias):
    hidden_states = np.asarray(hidden_states, dtype=np.float32)
    router_weight = np.asarray(router_weight, dtype=np.float32)
    router_bias = np.asarray(router_bias, dtype=np.float32)
    expert_bias = np.asarray(expert_bias, dtype=np.float32)
    Bv, Sv, Hv = hidden_states.shape
    assert (Bv, Sv, Hv) == (B, S, H), (Bv, Sv, Hv)

    flat = np.ascontiguousarray(hidden_states.reshape(T, H))
    rwt = np.ascontiguousarray(router_weight.T)            # [H, E]
    rb = np.ascontiguousarray(router_bias.reshape(E, 1))
    eb = np.ascontiguousarray(expert_bias)                 # [E, H]
    eye = np.eye(128, dtype=np.float32)

    nc = _get_nc()
    in_maps = []
    for c in range(N_CORES):
        in_maps.append({
            "x": flat[c * T_PC:(c + 1) * T_PC],
            "rwt": rwt,
            "eb": eb,
            "rb": rb,
            "idr": eye,
            "idf": eye,
        })
    res = run_bass_kernel_spmd(nc, in_maps, list(range(N_CORES)))
    out = np.concatenate([res.results[c]["out"] for c in range(N_CORES)], axis=0)
    return out.reshape(B, S, H)


if __name__ == "__main__":
    # quick self-run with random data
    rng = np.random.default_rng(0)
    hs = rng.standard_normal((B, S, H), dtype=np.float32)
    rw = rng.standard_normal((E, H), dtype=np.float32)
    rbv = np.zeros((E,), dtype=np.float32)
    ebv = (rng.standard_normal((E, H), dtype=np.float32) * 0.1).astype(np.float32)
    o = kernel(hidden_states=hs, router_weight=rw, router_bias=rbv, expert_bias=ebv)
    print("kernel out", o.shape, o.dtype, float(np.abs(o).mean()))
